# revision 54
# baseline (speedup 1.0000x reference)
"""Trainium2 Bass kernel for multi-head attention with RoPE (causal).

Problem: B=2, S=2048, D_MODEL=1024, N_HEADS=16, HEAD_DIM=64, theta=10000.
y = MHA(x) with per-head RoPE on Q/K, causal softmax, output projection.

Sharding over 8 NeuronCores: data-parallel on batch (2) x tensor-parallel on
heads (4 heads per core).  Each core computes a partial output projection
(row-parallel Wo); the host sums the 4 partials per batch.

On-chip dataflow (per core, all matmuls bf16 with fp32 PSUM accumulation):
  xT[e,s] (host-transposed) --PE--> QKV[s, 3*256] -> RoPE on DVE (host-permuted
  Wq/Wk rows make rotate-half contiguous) -> PE-transpose -> QT/KT[d,s]
  scoresT[k,q] = KT_j.T @ QT (2 heads packed in the 128-row PE array via
  tile_position) -> exp on ACT (scale=1/sqrt(dh), no max-subtraction: scores
  are ~N(0,1) so exp never overflows) -> causal mask multiply on diagonal
  tiles -> attn@V with a ones-augmented V column producing sumexp for free ->
  per-q reciprocal broadcast via a K=1 matmul -> normalize -> y = outT.T @ WoT.

Schedule notes (8 PSUM banks are the binding constraint):
  phase-1 pool: qkv (3x2 banks) + transpose staging (2x1); attention pool:
  scores/y-proj shared 3x2-bank tag + per-half attn@V accumulators (1+1,
  freed early via an SBUF drain).  Causal work is trimmed per k-tile to the
  valid q-columns; y-projection s-blocks are queued and interleaved into the
  next q-chunk's k-loop so the ACT-bound exp stream keeps the PE fed.
  Output copies are split DVE/ACT and DMA'd on two queues.

Measured (8x trn2 via axon, repeat-slope method): ~160-175 us per core;
cost-model sim 160.7 us; end-to-end relative error vs fp32 reference 7.6e-3.
"""

import sys
import numpy as np
import ml_dtypes
from contextlib import ExitStack

for _p in ("/opt/trn_rl_repo",):
    if _p not in sys.path:
        sys.path.insert(0, _p)

import concourse.bass as bass  # noqa: E402
import concourse.tile as tile  # noqa: E402
from concourse import bacc, mybir  # noqa: E402
from concourse.bass_utils import run_bass_kernel_spmd  # noqa: E402

F32 = mybir.dt.float32
BF16 = mybir.dt.bfloat16
AF = mybir.ActivationFunctionType
OP = mybir.AluOpType
bf16 = ml_dtypes.bfloat16

B, S_FULL, D, NH, DH = 2, 2048, 1024, 16, 64
THETA = 10000.0
N_CORES = 8
HPC = NH // (N_CORES // B)  # heads per core = 4
DLOC = HPC * DH             # 256 local head dims per core


def emit(nc, tc, ctx, io, S, repeat=1):
    """Emit the kernel body.  io: dict of DRAM APs."""
    NST = S // 128          # number of 128-row s-tiles
    NE = D // 128           # e (d_model) tiles = 8
    QC = min(512, S)        # q-chunk size
    NQC = S // QC
    KTB = 128               # k tile (partition dim of scoresT)

    consts = ctx.enter_context(tc.tile_pool(name="consts", bufs=1))
    work = ctx.enter_context(tc.tile_pool(name="work", bufs=3))
    etp = ctx.enter_context(tc.tile_pool(name="etp", bufs=8))
    yp = ctx.enter_context(tc.tile_pool(name="yp", bufs=3))

    # ---- persistent SBUF ----
    xt = consts.tile([128, NE, S], BF16, tag="xt")          # xT tiles
    wqkv = consts.tile([128, NE, 3 * DLOC], BF16, tag="wqkv")
    woT = consts.tile([128, 2, D], BF16, tag="woT")
    t1 = consts.tile([128, NST, 64], BF16, tag="t1")
    t2 = consts.tile([128, NST, 64], BF16, tag="t2")
    maskMB = consts.tile([128, 1024], BF16, tag="maskMB")
    ident = consts.tile([128, 128], BF16, tag="ident")
    ones64 = consts.tile([1, 64], BF16, tag="ones64")
    qkkt = consts.tile([128, 4, S], BF16, tag="qkkt")       # [Q01,Q23,K01,K23]
    vbuf = consts.tile([128, NST, HPC * 65], BF16, tag="vbuf")
    outT = consts.tile([128, 2, S], BF16, tag="outT")

    # ---- constant loads ----
    # two parallel DMA queues: sync carries xT (needed first), gpsimd the rest
    for e in range(NE):
        nc.gpsimd.dma_start(wqkv[:, e, :], io["wqkv"][e * 128:(e + 1) * 128, :])
    nc.gpsimd.dma_start(ident[:], io["ident"][:])
    for st in range(NST):
        nc.gpsimd.dma_start(t1[:, st, :], io["t1"][st * 128:(st + 1) * 128, :])
        nc.gpsimd.dma_start(t2[:, st, :], io["t2"][st * 128:(st + 1) * 128, :])
    nc.gpsimd.dma_start(maskMB[:], io["maskMB"][:])
    for i in range(2):
        nc.gpsimd.dma_start(woT[:, i, :], io["woT"][i * 128:(i + 1) * 128, :])
    nc.gpsimd.memset(ones64[:], 1.0)
    # ones columns interleaved into V stationary tiles
    nc.gpsimd.memset(
        vbuf[:].rearrange("p st (h c) -> p st h c", c=65)[:, :, :, 64:65], 1.0
    )

    for rep in range(repeat):
        for e in range(NE):
            nc.sync.dma_start(xt[:, e, :], io["xT"][e * 128:(e + 1) * 128, :])

        # ================= Phase 1: QKV projection + RoPE =================
        p1ctx = ExitStack()
        ps2 = p1ctx.enter_context(
            tc.tile_pool(name=f"ps_p1_{rep}", bufs=2, space="PSUM"))
        for st in range(NST):
            ss = slice(st * 128, (st + 1) * 128)
            qkv_ps = ps2.tile([128, 1024], F32, tag="qkv", bufs=3, name="qkv_ps")
            for e in range(NE):
                nc.tensor.matmul(qkv_ps[:, 0:512], xt[:, e, ss], wqkv[:, e, 0:512],
                                 start=(e == 0), stop=(e == NE - 1))
                nc.tensor.matmul(qkv_ps[:, 512:768], xt[:, e, ss], wqkv[:, e, 512:768],
                                 start=(e == 0), stop=(e == NE - 1))
            qkv_sb = work.tile([128, 512], BF16, tag="qkv_sb")
            nc.scalar.activation(qkv_sb[:], qkv_ps[:, 0:512], AF.Copy)

            # RoPE on q,k columns (cols 0:512); per 64-col head block the
            # first 32 cols are x1 (even dims), last 32 are x2 (odd dims).
            qk = qkv_sb[:].rearrange("p (h c) -> p h c", c=64)
            t1v = t1[:, st, :].rearrange("p (h c) -> p h c", c=64) \
                              .broadcast_to((128, 8, 64))
            t2v = t2[:, st, :].rearrange("p (h c) -> p h c", c=64) \
                              .broadcast_to((128, 8, 64))
            rp = work.tile([128, 8, 64], BF16, tag="ropeP")
            rq = work.tile([128, 8, 64], BF16, tag="ropeQ")
            ro = work.tile([128, 512], BF16, tag="ropeO")
            rov = ro[:].rearrange("p (h c) -> p h c", c=64)
            nc.vector.tensor_tensor(rp[:], qk, t1v, OP.mult)       # [x1*c, x2*s]
            nc.vector.tensor_tensor(rq[:], qk, t2v, OP.mult)       # [x1*s, x2*c]
            nc.vector.tensor_tensor(rov[:, :, 0:32], rp[:, :, 0:32],
                                    rp[:, :, 32:64], OP.subtract)
            nc.vector.tensor_tensor(rov[:, :, 32:64], rq[:, :, 0:32],
                                    rq[:, :, 32:64], OP.add)

            # V -> interleaved stationary buffer (65-wide per head, ones col)
            nc.vector.tensor_copy(
                vbuf[:, st, :].rearrange("p (h c) -> p h c", c=65)[:, :, 0:64],
                qkv_ps[:, 512:768].rearrange("p (h c) -> p h c", c=64))

            # transpose rope output into [d, s] layout
            tps = ps2.tile([128, 512], BF16, tag="tps", bufs=2, name="tps")
            for blk in range(4):
                bs = slice(blk * 128, (blk + 1) * 128)
                nc.tensor.transpose(tps[:, bs], ro[:, bs], ident[:])
            nc.vector.tensor_copy(
                qkkt[:, :, ss],
                tps[:].rearrange("p (b c) -> p b c", c=128))

        p1ctx.close()

        # ================= Phase 2: attention (pair-major) =================
        p2ctx = ExitStack()
        ps2 = p2ctx.enter_context(
            tc.tile_pool(name=f"ps_p2_{rep}", bufs=2, space="PSUM"))
        y_queue = []

        def emit_y_block(sb):
            sbs = slice(sb * 128, (sb + 1) * 128)
            yps = ps2.tile([128, D], F32, tag="sc", bufs=3, name="yps")
            for hp in range(2):
                for n in range(D // 512):
                    ns = slice(n * 512, (n + 1) * 512)
                    nc.tensor.matmul(yps[:, ns], outT[:, hp, sbs],
                                     woT[:, hp, ns],
                                     start=(hp == 0), stop=(hp == 1))
            ysb = yp.tile([128, D], F32, tag="ysb")
            nc.vector.tensor_copy(ysb[:, 0:D // 2], yps[:, 0:D // 2])
            nc.scalar.activation(ysb[:, D // 2:D], yps[:, D // 2:D], AF.Copy)
            nc.gpsimd.dma_start(io["yp"][sbs, 0:D // 2], ysb[:, 0:D // 2])
            nc.sync.dma_start(io["yp"][sbs, D // 2:D], ysb[:, D // 2:D])

        for p in range(2):
            for qc in range(NQC):
                qlo = qc * QC
                jmax = ((qc + 1) * QC) // KTB - 1
                po = [ps2.tile([65, QC], F32, tag=f"po{half}", bufs=1,
                               name=f"po{half}") for half in range(2)]
                for j in range(jmax + 1):
                    js = slice(j * KTB, (j + 1) * KTB)
                    o = max(0, j * KTB - qlo)   # diagonal column offset
                    ncols = QC - o
                    qs = slice(qlo + o, qlo + QC)
                    sc = ps2.tile([128, 2 * QC], F32, tag="sc", bufs=3)
                    scv = sc[:].rearrange("p (t q) -> p t q", t=2)
                    nc.tensor.matmul(scv[:, 0, o:QC], qkkt[0:64, 2 + p, js],
                                     qkkt[0:64, p, qs], start=True, stop=True,
                                     tile_position=(0, 0))
                    nc.tensor.matmul(scv[:, 1, o:QC], qkkt[64:128, 2 + p, js],
                                     qkkt[64:128, p, qs], start=True, stop=True,
                                     tile_position=(64, 0))
                    et = etp.tile([128, 2, QC], BF16, tag="et")
                    nc.scalar.activation(et[:, :, o:QC], scv[:, :, o:QC], AF.Exp,
                                         scale=float(1.0 / np.sqrt(DH)))
                    if o > 0 or j * KTB == qlo:  # diagonal tile: causal mask
                        nc.vector.tensor_tensor(
                            et[:, :, o:QC], et[:, :, o:QC],
                            maskMB[:, 384:384 + ncols].unsqueeze(1)
                                  .broadcast_to((128, 2, ncols)),
                            OP.mult)
                    for half in range(2):
                        h = 2 * p + half
                        nc.tensor.matmul(po[half][:, o:QC],
                                         vbuf[:, j, h * 65:h * 65 + 65],
                                         et[:, half, o:QC],
                                         start=(j == 0), stop=(j == jmax))
                    if y_queue and j % 2 == 1:
                        emit_y_block(y_queue.pop(0))

                # normalization: divide by sumexp (row 64 of po)
                for half in range(2):
                    posb = work.tile([65, QC], F32, tag="posb")
                    nc.vector.tensor_copy(posb[:], po[half][:])
                    r_bf = work.tile([1, QC], BF16, tag="r_bf")
                    with nc.allow_low_precision("softmax denom in bf16"):
                        nc.vector.reciprocal(r_bf[:], posb[64:65, :])
                    pr = ps2.tile([64, QC], F32, tag=f"po{half}", bufs=1, name="pr")
                    nc.tensor.matmul(pr[:], ones64[:], r_bf[:],
                                     start=True, stop=True)
                    nc.vector.tensor_tensor(
                        outT[64 * half:64 * half + 64, p, qlo:qlo + QC],
                        posb[0:64, :], pr[:], OP.mult)

                # ==== Phase 3: queue output projection; its s-blocks are
                # interleaved into later j-loops to spread PSUM slot demand ===
                if p == 1:
                    y_queue.extend(range(qc * (QC // 128), (qc + 1) * (QC // 128)))
            if p == 1:
                while y_queue:
                    emit_y_block(y_queue.pop(0))
        p2ctx.close()


def build_program(S=S_FULL, repeat=1):
    nc = bacc.Bacc("TRN2", target_bir_lowering=False, debug=False,
                   num_devices=N_CORES)
    io = {
        "xT": nc.dram_tensor("xT", [D, S], BF16, kind="ExternalInput").ap(),
        "wqkv": nc.dram_tensor("wqkv", [D, 3 * DLOC], BF16,
                               kind="ExternalInput").ap(),
        "woT": nc.dram_tensor("woT", [DLOC, D], BF16, kind="ExternalInput").ap(),
        "t1": nc.dram_tensor("t1", [S, 64], BF16, kind="ExternalInput").ap(),
        "t2": nc.dram_tensor("t2", [S, 64], BF16, kind="ExternalInput").ap(),
        "maskMB": nc.dram_tensor("maskMB", [128, 1024], BF16,
                                 kind="ExternalInput").ap(),
        "ident": nc.dram_tensor("ident", [128, 128], BF16,
                                kind="ExternalInput").ap(),
        "yp": nc.dram_tensor("yp", [S, D], F32, kind="ExternalOutput").ap(),
    }
    with tile.TileContext(nc) as tc, ExitStack() as ctx:
        emit(nc, tc, ctx, io, S, repeat=repeat)
    nc.compile()
    return nc


# head-dim permutation: even dims then odd dims (rotate-half-friendly)
_PERM = np.concatenate([np.arange(0, DH, 2), np.arange(1, DH, 2)])


def host_inputs(x, token_positions, Wq, Wk, Wv, Wo, S=S_FULL):
    """Build per-core input maps (host-side prep is free)."""
    pos = np.asarray(token_positions).astype(np.float64)
    inv_freq = THETA ** (-np.arange(32, dtype=np.float64) / 32.0)
    ang = pos[:, None] * inv_freq[None, :]          # [S, 32]
    t1 = np.concatenate([np.cos(ang), np.sin(ang)], axis=1).astype(bf16)
    t2 = np.concatenate([np.sin(ang), np.cos(ang)], axis=1).astype(bf16)

    k_idx = np.arange(128)[:, None]
    m_idx = np.arange(1024)[None, :]
    maskMB = (m_idx >= k_idx + 384).astype(np.float32).astype(bf16)
    ident = np.eye(128, dtype=np.float32).astype(bf16)

    x = np.asarray(x, dtype=np.float32)
    Wq = np.asarray(Wq, dtype=np.float32)
    Wk = np.asarray(Wk, dtype=np.float32)
    Wv = np.asarray(Wv, dtype=np.float32)
    Wo = np.asarray(Wo, dtype=np.float32)

    xT = [np.ascontiguousarray(x[b, :S].T).astype(bf16) for b in range(B)]
    in_maps = []
    for c in range(N_CORES):
        b, g = divmod(c, N_CORES // B)
        heads = range(HPC * g, HPC * (g + 1))
        wq_rows = np.concatenate(
            [Wq[h * DH:(h + 1) * DH][_PERM] for h in heads])     # [256, 1024]
        wk_rows = np.concatenate(
            [Wk[h * DH:(h + 1) * DH][_PERM] for h in heads])
        wv_rows = np.concatenate([Wv[h * DH:(h + 1) * DH] for h in heads])
        wqkv = np.ascontiguousarray(
            np.concatenate([wq_rows, wk_rows, wv_rows]).T).astype(bf16)
        woT_g = np.ascontiguousarray(
            np.concatenate([Wo[:, h * DH:(h + 1) * DH].T for h in heads])
        ).astype(bf16)
        in_maps.append({
            "xT": xT[b], "wqkv": wqkv, "woT": woT_g,
            "t1": t1[:S], "t2": t2[:S], "maskMB": maskMB, "ident": ident,
        })
    return in_maps


_PROGRAM_CACHE = {}


def kernel(x, token_positions, Wq, Wk, Wv, Wo):
    if "nc" not in _PROGRAM_CACHE:
        _PROGRAM_CACHE["nc"] = build_program()
    nc = _PROGRAM_CACHE["nc"]
    in_maps = host_inputs(x, token_positions, Wq, Wk, Wv, Wo)
    res = run_bass_kernel_spmd(nc, in_maps, list(range(N_CORES)))
    parts = [res.results[c]["yp"].astype(np.float64) for c in range(N_CORES)]
    gpb = N_CORES // B
    y = np.stack([sum(parts[b * gpb:(b + 1) * gpb]) for b in range(B)])
    return y.astype(np.float32)


# revision 57
# speedup vs baseline: 1.2148x; 1.2148x over previous
"""Trainium2 Bass kernel for multi-head attention with RoPE (causal).

Problem: B=2, S=2048, D_MODEL=1024, N_HEADS=16, HEAD_DIM=64, theta=10000.
y = MHA(x) with per-head RoPE on Q/K, causal softmax, output projection.

Sharding over 8 NeuronCores: data-parallel on batch (2) x tensor-parallel on
heads (4 heads per core).  Each core computes a partial output projection
(row-parallel Wo); the host sums the 4 partials per batch.

On-chip dataflow (per core, all matmuls bf16 with fp32 PSUM accumulation):
  xT[e,s] (host-transposed) --PE--> QKV[s, 3*256] -> RoPE on DVE (host-permuted
  Wq/Wk rows make rotate-half contiguous) -> PE-transpose -> QT/KT[d,s]
  scoresT[k,q] = KT_j.T @ QT (2 heads packed in the 128-row PE array via
  tile_position) -> exp on ACT (scale=1/sqrt(dh), no max-subtraction: scores
  are ~N(0,1) so exp never overflows) -> causal mask multiply on diagonal
  tiles -> attn@V with a ones-augmented V column producing sumexp for free ->
  per-q reciprocal broadcast via a K=1 matmul -> normalize -> y = outT.T @ WoT.

Schedule notes (8 PSUM banks are the binding constraint):
  phase-1 pool: qkv (3x2 banks) + transpose staging (2x1); attention pool:
  scores/y-proj shared 3x2-bank tag + per-half attn@V accumulators (1+1,
  freed early via an SBUF drain).  Causal work is trimmed per k-tile to the
  valid q-columns; y-projection s-blocks are queued and interleaved into the
  next q-chunk's k-loop so the ACT-bound exp stream keeps the PE fed.
  Output copies are split DVE/ACT and DMA'd on two queues.

Measured (8x trn2 via axon, repeat-slope method): ~160-175 us per core;
cost-model sim 160.7 us; end-to-end relative error vs fp32 reference 7.6e-3.
"""

import sys
import numpy as np
import ml_dtypes
from contextlib import ExitStack

for _p in ("/opt/trn_rl_repo",):
    if _p not in sys.path:
        sys.path.insert(0, _p)

import concourse.bass as bass  # noqa: E402
import concourse.tile as tile  # noqa: E402
from concourse import bacc, mybir  # noqa: E402
from concourse.bass_utils import run_bass_kernel_spmd  # noqa: E402

F32 = mybir.dt.float32
BF16 = mybir.dt.bfloat16
AF = mybir.ActivationFunctionType
OP = mybir.AluOpType
bf16 = ml_dtypes.bfloat16

B, S_FULL, D, NH, DH = 2, 2048, 1024, 16, 64
THETA = 10000.0
N_CORES = 8
HPC = NH // (N_CORES // B)  # heads per core = 4
DLOC = HPC * DH             # 256 local head dims per core


def emit(nc, tc, ctx, io, S, repeat=1):
    """Emit the kernel body.  io: dict of DRAM APs."""
    NST = S // 128          # number of 128-row s-tiles
    NE = D // 128           # e (d_model) tiles = 8
    QC = min(512, S)        # q-chunk size
    NQC = S // QC
    KTB = 128               # k tile (partition dim of scoresT)

    consts = ctx.enter_context(tc.tile_pool(name="consts", bufs=1))
    work = ctx.enter_context(tc.tile_pool(name="work", bufs=3))
    etp = ctx.enter_context(tc.tile_pool(name="etp", bufs=8))
    yp = ctx.enter_context(tc.tile_pool(name="yp", bufs=3))

    # ---- persistent SBUF ----
    xt = consts.tile([128, NE, S], BF16, tag="xt")          # xT tiles
    wqkv = consts.tile([128, NE, 3 * DLOC], BF16, tag="wqkv")
    woT = consts.tile([128, 2, D], BF16, tag="woT")
    t1 = consts.tile([128, NST, 64], BF16, tag="t1")
    t2 = consts.tile([128, NST, 64], BF16, tag="t2")
    maskMB = consts.tile([128, 1024], BF16, tag="maskMB")
    ident = consts.tile([128, 128], BF16, tag="ident")
    ones64 = consts.tile([1, 64], BF16, tag="ones64")
    qkkt = consts.tile([128, 4, S], BF16, tag="qkkt")       # [Q01,Q23,K01,K23]
    vbuf = consts.tile([128, NST, HPC * 65], BF16, tag="vbuf")
    outT = consts.tile([128, 2, S], BF16, tag="outT")

    # ---- constant loads ----
    # two parallel DMA queues: sync carries xT (needed first), gpsimd the rest
    for e in range(NE):
        nc.gpsimd.dma_start(wqkv[:, e, :], io["wqkv"][e * 128:(e + 1) * 128, :])
    nc.gpsimd.dma_start(ident[:], io["ident"][:])
    for st in range(NST):
        nc.gpsimd.dma_start(t1[:, st, :], io["t1"][st * 128:(st + 1) * 128, :])
        nc.gpsimd.dma_start(t2[:, st, :], io["t2"][st * 128:(st + 1) * 128, :])
    nc.gpsimd.dma_start(maskMB[:], io["maskMB"][:])
    for i in range(2):
        nc.gpsimd.dma_start(woT[:, i, :], io["woT"][i * 128:(i + 1) * 128, :])
    nc.gpsimd.memset(ones64[:], 1.0)
    # ones columns interleaved into V stationary tiles
    nc.gpsimd.memset(
        vbuf[:].rearrange("p st (h c) -> p st h c", c=65)[:, :, :, 64:65], 1.0
    )

    for rep in range(repeat):
        # split the x load over three otherwise-idle DMA queues so the first
        # s-tile's 8-slice accumulation isn't paced by one serial queue
        xt_q = [nc.sync, nc.sync, nc.sync, nc.sync, nc.scalar, nc.scalar,
                nc.scalar, nc.scalar]
        # tiny first slice so the very first Ldweights can start early
        nc.sync.dma_start(xt[:, 0, 0:128], io["xT"][0:128, 0:128])
        for e in range(NE):
            lo = 128 if e == 0 else 0
            xt_q[e].dma_start(xt[:, e, lo:S], io["xT"][e * 128:(e + 1) * 128, lo:S])

        # ================= Phase 1: QKV projection + RoPE =================
        p1ctx = ExitStack()
        ps2 = p1ctx.enter_context(
            tc.tile_pool(name=f"ps_p1_{rep}", bufs=2, space="PSUM"))
        for st in range(NST):
            ss = slice(st * 128, (st + 1) * 128)
            qkv_ps = ps2.tile([128, 1024], F32, tag="qkv", bufs=3, name="qkv_ps")
            for e in range(NE):
                nc.tensor.matmul(qkv_ps[:, 0:512], xt[:, e, ss], wqkv[:, e, 0:512],
                                 start=(e == 0), stop=(e == NE - 1))
                nc.tensor.matmul(qkv_ps[:, 512:768], xt[:, e, ss], wqkv[:, e, 512:768],
                                 start=(e == 0), stop=(e == NE - 1))
            qkv_sb = work.tile([128, 512], BF16, tag="qkv_sb")
            nc.scalar.activation(qkv_sb[:], qkv_ps[:, 0:512], AF.Copy)

            # RoPE on q,k columns (cols 0:512); per 64-col head block the
            # first 32 cols are x1 (even dims), last 32 are x2 (odd dims).
            qk = qkv_sb[:].rearrange("p (h c) -> p h c", c=64)
            t1v = t1[:, st, :].rearrange("p (h c) -> p h c", c=64) \
                              .broadcast_to((128, 8, 64))
            t2v = t2[:, st, :].rearrange("p (h c) -> p h c", c=64) \
                              .broadcast_to((128, 8, 64))
            rp = work.tile([128, 8, 64], BF16, tag="ropeP")
            rq = work.tile([128, 8, 64], BF16, tag="ropeQ")
            ro = work.tile([128, 512], BF16, tag="ropeO")
            rov = ro[:].rearrange("p (h c) -> p h c", c=64)
            nc.vector.tensor_tensor(rp[:], qk, t1v, OP.mult)       # [x1*c, x2*s]
            nc.vector.tensor_tensor(rq[:], qk, t2v, OP.mult)       # [x1*s, x2*c]
            nc.vector.tensor_tensor(rov[:, :, 0:32], rp[:, :, 0:32],
                                    rp[:, :, 32:64], OP.subtract)
            nc.vector.tensor_tensor(rov[:, :, 32:64], rq[:, :, 0:32],
                                    rq[:, :, 32:64], OP.add)

            # V -> interleaved stationary buffer (65-wide per head, ones col)
            nc.vector.tensor_copy(
                vbuf[:, st, :].rearrange("p (h c) -> p h c", c=65)[:, :, 0:64],
                qkv_ps[:, 512:768].rearrange("p (h c) -> p h c", c=64))

            # transpose rope output into [d, s] layout
            tps = ps2.tile([128, 512], BF16, tag="tps", bufs=2, name="tps")
            for blk in range(4):
                bs = slice(blk * 128, (blk + 1) * 128)
                nc.tensor.transpose(tps[:, bs], ro[:, bs], ident[:])
            nc.vector.tensor_copy(
                qkkt[:, :, ss],
                tps[:].rearrange("p (b c) -> p b c", c=128))

        p1ctx.close()

        # ================= Phase 2: attention (pair-major) =================
        p2ctx = ExitStack()
        ps2 = p2ctx.enter_context(
            tc.tile_pool(name=f"ps_p2_{rep}", bufs=2, space="PSUM"))
        y_queue = []

        def emit_y_block(sb):
            sbs = slice(sb * 128, (sb + 1) * 128)
            yps = ps2.tile([128, D], F32, tag="sc", bufs=3, name="yps")
            for hp in range(2):
                for n in range(D // 512):
                    ns = slice(n * 512, (n + 1) * 512)
                    nc.tensor.matmul(yps[:, ns], outT[:, hp, sbs],
                                     woT[:, hp, ns],
                                     start=(hp == 0), stop=(hp == 1))
            ysb = yp.tile([128, D], F32, tag="ysb")
            nc.vector.tensor_copy(ysb[:, 0:D // 2], yps[:, 0:D // 2])
            nc.scalar.activation(ysb[:, D // 2:D], yps[:, D // 2:D], AF.Copy)
            nc.gpsimd.dma_start(io["yp"][sbs, 0:D // 2], ysb[:, 0:D // 2])
            nc.sync.dma_start(io["yp"][sbs, D // 2:D], ysb[:, D // 2:D])

        for p in range(2):
            for qc in range(NQC):
                qlo = qc * QC
                jmax = ((qc + 1) * QC) // KTB - 1
                po = [ps2.tile([65, QC], F32, tag=f"po{half}", bufs=1,
                               name=f"po{half}") for half in range(2)]
                for j in range(jmax + 1):
                    js = slice(j * KTB, (j + 1) * KTB)
                    o = max(0, j * KTB - qlo)   # diagonal column offset
                    ncols = QC - o
                    qs = slice(qlo + o, qlo + QC)
                    sc = ps2.tile([128, 2 * QC], F32, tag="sc", bufs=3)
                    scv = sc[:].rearrange("p (t q) -> p t q", t=2)
                    nc.tensor.matmul(scv[:, 0, o:QC], qkkt[0:64, 2 + p, js],
                                     qkkt[0:64, p, qs], start=True, stop=True,
                                     tile_position=(0, 0))
                    nc.tensor.matmul(scv[:, 1, o:QC], qkkt[64:128, 2 + p, js],
                                     qkkt[64:128, p, qs], start=True, stop=True,
                                     tile_position=(64, 0))
                    et = etp.tile([128, 2, QC], BF16, tag="et")
                    nc.scalar.activation(et[:, :, o:QC], scv[:, :, o:QC], AF.Exp,
                                         scale=float(1.0 / np.sqrt(DH)))
                    if o > 0 or j * KTB == qlo:  # diagonal tile: causal mask
                        nc.vector.tensor_tensor(
                            et[:, :, o:QC], et[:, :, o:QC],
                            maskMB[:, 384:384 + ncols].unsqueeze(1)
                                  .broadcast_to((128, 2, ncols)),
                            OP.mult)
                    for half in range(2):
                        h = 2 * p + half
                        nc.tensor.matmul(po[half][:, o:QC],
                                         vbuf[:, j, h * 65:h * 65 + 65],
                                         et[:, half, o:QC],
                                         start=(j == 0), stop=(j == jmax))
                    if y_queue and j % 2 == 1:
                        emit_y_block(y_queue.pop(0))

                # normalization: divide by sumexp (row 64 of po)
                for half in range(2):
                    posb = work.tile([65, QC], F32, tag="posb")
                    nc.vector.tensor_copy(posb[:], po[half][:])
                    r_bf = work.tile([1, QC], BF16, tag="r_bf")
                    with nc.allow_low_precision("softmax denom in bf16"):
                        nc.vector.reciprocal(r_bf[:], posb[64:65, :])
                    pr = ps2.tile([64, QC], F32, tag=f"po{half}", bufs=1, name="pr")
                    nc.tensor.matmul(pr[:], ones64[:], r_bf[:],
                                     start=True, stop=True)
                    nc.vector.tensor_tensor(
                        outT[64 * half:64 * half + 64, p, qlo:qlo + QC],
                        posb[0:64, :], pr[:], OP.mult)

                # ==== Phase 3: queue output projection; its s-blocks are
                # interleaved into later j-loops to spread PSUM slot demand ===
                if p == 1:
                    y_queue.extend(range(qc * (QC // 128), (qc + 1) * (QC // 128)))
            if p == 1:
                while y_queue:
                    emit_y_block(y_queue.pop(0))
        p2ctx.close()


def build_program(S=S_FULL, repeat=1):
    nc = bacc.Bacc("TRN2", target_bir_lowering=False, debug=False,
                   num_devices=N_CORES)
    io = {
        "xT": nc.dram_tensor("xT", [D, S], BF16, kind="ExternalInput").ap(),
        "wqkv": nc.dram_tensor("wqkv", [D, 3 * DLOC], BF16,
                               kind="ExternalInput").ap(),
        "woT": nc.dram_tensor("woT", [DLOC, D], BF16, kind="ExternalInput").ap(),
        "t1": nc.dram_tensor("t1", [S, 64], BF16, kind="ExternalInput").ap(),
        "t2": nc.dram_tensor("t2", [S, 64], BF16, kind="ExternalInput").ap(),
        "maskMB": nc.dram_tensor("maskMB", [128, 1024], BF16,
                                 kind="ExternalInput").ap(),
        "ident": nc.dram_tensor("ident", [128, 128], BF16,
                                kind="ExternalInput").ap(),
        "yp": nc.dram_tensor("yp", [S, D], F32, kind="ExternalOutput").ap(),
    }
    with tile.TileContext(nc) as tc, ExitStack() as ctx:
        emit(nc, tc, ctx, io, S, repeat=repeat)
    nc.compile()
    return nc


# head-dim permutation: even dims then odd dims (rotate-half-friendly)
_PERM = np.concatenate([np.arange(0, DH, 2), np.arange(1, DH, 2)])


def host_inputs(x, token_positions, Wq, Wk, Wv, Wo, S=S_FULL):
    """Build per-core input maps (host-side prep is free)."""
    pos = np.asarray(token_positions).astype(np.float64)
    inv_freq = THETA ** (-np.arange(32, dtype=np.float64) / 32.0)
    ang = pos[:, None] * inv_freq[None, :]          # [S, 32]
    t1 = np.concatenate([np.cos(ang), np.sin(ang)], axis=1).astype(bf16)
    t2 = np.concatenate([np.sin(ang), np.cos(ang)], axis=1).astype(bf16)

    k_idx = np.arange(128)[:, None]
    m_idx = np.arange(1024)[None, :]
    maskMB = (m_idx >= k_idx + 384).astype(np.float32).astype(bf16)
    ident = np.eye(128, dtype=np.float32).astype(bf16)

    x = np.asarray(x, dtype=np.float32)
    Wq = np.asarray(Wq, dtype=np.float32)
    Wk = np.asarray(Wk, dtype=np.float32)
    Wv = np.asarray(Wv, dtype=np.float32)
    Wo = np.asarray(Wo, dtype=np.float32)

    xT = [np.ascontiguousarray(x[b, :S].T).astype(bf16) for b in range(B)]
    in_maps = []
    for c in range(N_CORES):
        b, g = divmod(c, N_CORES // B)
        heads = range(HPC * g, HPC * (g + 1))
        wq_rows = np.concatenate(
            [Wq[h * DH:(h + 1) * DH][_PERM] for h in heads])     # [256, 1024]
        wk_rows = np.concatenate(
            [Wk[h * DH:(h + 1) * DH][_PERM] for h in heads])
        wv_rows = np.concatenate([Wv[h * DH:(h + 1) * DH] for h in heads])
        wqkv = np.ascontiguousarray(
            np.concatenate([wq_rows, wk_rows, wv_rows]).T).astype(bf16)
        woT_g = np.ascontiguousarray(
            np.concatenate([Wo[:, h * DH:(h + 1) * DH].T for h in heads])
        ).astype(bf16)
        in_maps.append({
            "xT": xT[b], "wqkv": wqkv, "woT": woT_g,
            "t1": t1[:S], "t2": t2[:S], "maskMB": maskMB, "ident": ident,
        })
    return in_maps


_PROGRAM_CACHE = {}


def kernel(x, token_positions, Wq, Wk, Wv, Wo):
    if "nc" not in _PROGRAM_CACHE:
        _PROGRAM_CACHE["nc"] = build_program()
    nc = _PROGRAM_CACHE["nc"]
    in_maps = host_inputs(x, token_positions, Wq, Wk, Wv, Wo)
    res = run_bass_kernel_spmd(nc, in_maps, list(range(N_CORES)))
    parts = [res.results[c]["yp"].astype(np.float64) for c in range(N_CORES)]
    gpb = N_CORES // B
    y = np.stack([sum(parts[b * gpb:(b + 1) * gpb]) for b in range(B)])
    return y.astype(np.float32)
